# revision 2
# baseline (speedup 1.0000x reference)
"""Bass/Trainium2 kernel for cubic B-spline encoding — scatter architecture, J=10.

Per group: 1280 points = 128 partitions x 10 points.
  - DVE computes per-point features on [128, 30] tiles:
    xs=(x+1)*30.5, xh=xs-0.5, u=xs-idx, u^2, u^3
  - ScalarE casts xh -> int16 (round == floor(xs)) and back (idx feature).
  - PE transposes the [128, 150] feature tile in two halves and runs 3
    accumulating matmuls that emit, per (point, dim): the 4 spline
    coefficients c_q(u), the x passthrough, and the int16 scatter indices
    idx + rowbase + q (exact integers in fp32).
  - GPSIMD local_scatter zero-fills the [128, 10*195] fp16 out tile and
    places the 15 values per point.  No dense 64-wide elementwise work.
  - One contiguous DMA per group writes 10 rows x 195 fp16 per partition.
Output is fp16 in HBM (error ~2e-4 << 2e-2 tolerance); host upcasts to f32.
"""

import math
import os
import sys
from contextlib import ExitStack

import numpy as np

for _p in ("/opt/trn_rl_repo", "/root/.axon_site/_ro/trn_rl_repo"):
    if os.path.isdir(_p) and _p not in sys.path:
        sys.path.insert(0, _p)

import concourse.bass as bass  # noqa: E402,F401
import concourse.tile as tile  # noqa: E402
from concourse import bacc, mybir  # noqa: E402
from concourse import bass_utils  # noqa: E402

F32 = mybir.dt.float32
F16 = mybir.dt.float16
I16 = mybir.dt.int16

N_CORES = 8
D = 3
K = 64
ROW = D * (1 + K)          # 195 outputs per point
J = 10                     # points per partition per group
GROUP = 128 * J            # 1280 points per group
NB = J * D                 # 30 (j,d) blocks per partition per group
NF = 5                     # feature planes: u, u2, u3, x, idx
FROWS = NF * NB            # 150 feature rows (split 75/75 for transpose)
FH = 75                    # transpose half
NI = NB * 5                # 150 scatter slots (c0..c3 + x per (j,d))
NE = J * ROW               # 1950 out elems per partition (cap: 2047)
SCALE = (K - 3) / 2.0      # 30.5
MAX_SG = 12                # groups per input-DMA supergroup

PU, PU2, PU3, PX, PIDX = range(NF)


def _host_consts():
    ident = np.eye(128, dtype=np.float32)
    E = np.zeros((FROWS, 2 * NI), dtype=np.float32)
    crow = np.zeros((1, 2 * NI), dtype=np.float32)
    ones = np.ones((1, 128), dtype=np.float32)
    CQ = np.array([
        [1 / 6, -3 / 6, 3 / 6, -1 / 6],   # c0 = (1-u)^3/6
        [4 / 6, 0.0, -1.0, 0.5],          # c1 = (3u^3-6u^2+4)/6
        [1 / 6, 3 / 6, 3 / 6, -0.5],      # c2 = (-3u^3+3u^2+3u+1)/6
        [0.0, 0.0, 0.0, 1 / 6],           # c3 = u^3/6
    ], dtype=np.float64)
    for j in range(J):
        for d in range(D):
            m = j * D + d
            base = ROW * j + (1 + K) * d
            for q in range(4):
                c = m * 5 + q
                crow[0, c] = CQ[q, 0]
                E[PU * NB + m, c] = CQ[q, 1]
                E[PU2 * NB + m, c] = CQ[q, 2]
                E[PU3 * NB + m, c] = CQ[q, 3]
                E[PIDX * NB + m, NI + c] = 1.0
                crow[0, NI + c] = base + 1 + q
            cx = m * 5 + 4
            E[PX * NB + m, cx] = 1.0
            crow[0, NI + cx] = base
    return ident, E[:FH], E[FH:], crow, ones


def _split_supergroups(n_groups):
    sizes = []
    left = n_groups
    while left > 0:
        g = min(MAX_SG, left)
        sizes.append(g)
        left -= g
    return sizes


def build_program(npad):
    assert npad % GROUP == 0
    n_groups = npad // GROUP
    nc = bacc.Bacc("TRN2", target_bir_lowering=False, debug=False,
                   num_devices=N_CORES)
    x_d = nc.dram_tensor("x", [npad, D], F32, kind="ExternalInput").ap()
    out_d = nc.dram_tensor("out", [npad, ROW], F16,
                           kind="ExternalOutput").ap()
    ident_d = nc.dram_tensor("ident", [128, 128], F32,
                             kind="ExternalInput").ap()
    e1_d = nc.dram_tensor("e1", [FH, 2 * NI], F32, kind="ExternalInput").ap()
    e2_d = nc.dram_tensor("e2", [FROWS - FH, 2 * NI], F32,
                          kind="ExternalInput").ap()
    crow_d = nc.dram_tensor("crow", [1, 2 * NI], F32,
                            kind="ExternalInput").ap()
    ones_d = nc.dram_tensor("ones", [1, 128], F32, kind="ExternalInput").ap()

    AL = mybir.AluOpType

    with tile.TileContext(nc) as tc, ExitStack() as ctx:
        cpool = ctx.enter_context(tc.tile_pool(name="const", bufs=1))
        ident_t = cpool.tile([128, 128], F32, tag="ident")
        nc.sync.dma_start(ident_t[:], ident_d[:])
        e1_t = cpool.tile([FH, 2 * NI], F32, tag="e1")
        nc.sync.dma_start(e1_t[:], e1_d[:])
        e2_t = cpool.tile([FROWS - FH, 2 * NI], F32, tag="e2")
        nc.sync.dma_start(e2_t[:], e2_d[:])
        crow_t = cpool.tile([1, 2 * NI], F32, tag="crow")
        nc.sync.dma_start(crow_t[:], crow_d[:])
        ones_t = cpool.tile([1, 128], F32, tag="ones")
        nc.sync.dma_start(ones_t[:], ones_d[:])

        xin_p = ctx.enter_context(tc.tile_pool(name="xin", bufs=2))
        ft_p = ctx.enter_context(tc.tile_pool(name="ft", bufs=3))
        xs_p = ctx.enter_context(tc.tile_pool(name="xs", bufs=3))
        ii_p = ctx.enter_context(tc.tile_pool(name="ii", bufs=3))
        xT_p = ctx.enter_context(tc.tile_pool(name="xT", bufs=4))
        di_p = ctx.enter_context(tc.tile_pool(name="di", bufs=3))
        out_p = ctx.enter_context(tc.tile_pool(name="out", bufs=4))
        psT_p = ctx.enter_context(tc.tile_pool(name="psT", bufs=2,
                                               space="PSUM"))
        psG_p = ctx.enter_context(tc.tile_pool(name="psG", bufs=2,
                                               space="PSUM"))

        g0 = 0
        for G in _split_supergroups(n_groups):
            b0 = g0 * GROUP
            x_sl = x_d[b0:b0 + GROUP * G, :].rearrange(
                "(p k) d -> p (k d)", p=128)
            out_sl = out_d[b0:b0 + GROUP * G, :].rearrange(
                "(p g j) f -> g p (j f)", p=128, j=J)
            xin = xin_p.tile([128, G * NB], F32, tag="xin", name="xin")
            nc.sync.dma_start(xin[:], x_sl)

            for g in range(G):
                x_g = xin[:, g * NB:(g + 1) * NB]       # [128, 30]
                ft = ft_p.tile([128, FROWS], F32, tag="ft", name="ft")
                u = ft[:, PU * NB:(PU + 1) * NB]
                u2 = ft[:, PU2 * NB:(PU2 + 1) * NB]
                u3 = ft[:, PU3 * NB:(PU3 + 1) * NB]
                xf = ft[:, PX * NB:(PX + 1) * NB]
                idxf = ft[:, PIDX * NB:(PIDX + 1) * NB]

                xs = xs_p.tile([128, NB], F32, tag="xs", name="xs")
                # xs = (x + 1) * 30.5  (same op order as the reference)
                nc.vector.tensor_scalar(xs[:], x_g, 1.0, SCALE,
                                        AL.add, AL.mult)
                # idx = floor(xs) via round-to-nearest int16 cast of xs-0.5
                xh = xs_p.tile([128, NB], F32, tag="xh", name="xh")
                nc.vector.tensor_scalar(xh[:], xs[:], 0.5, 1.0,
                                        AL.subtract, AL.mult)
                idx_i = ii_p.tile([128, NB], I16, tag="ii", name="ii")
                nc.scalar.copy(idx_i[:], xh[:])
                nc.scalar.copy(idxf, idx_i[:])
                nc.vector.tensor_tensor(u, xs[:], idxf, AL.subtract)
                nc.vector.tensor_tensor(u2, u, u, AL.mult)
                nc.vector.tensor_tensor(u3, u2, u, AL.mult)
                nc.vector.tensor_copy(xf, x_g)

                psT1 = psT_p.tile([FH, 128], F32, tag="psT1", name="psT1")
                nc.tensor.transpose(psT1[:], ft[:, :FH], ident_t[:])
                psT2 = psT_p.tile([FROWS - FH, 128], F32, tag="psT2",
                                  name="psT2")
                nc.tensor.transpose(psT2[:], ft[:, FH:], ident_t[:])
                xT1 = xT_p.tile([FH, 128], F32, tag="xT1", name="xT1")
                nc.scalar.copy(xT1[:], psT1[:])
                xT2 = xT_p.tile([FROWS - FH, 128], F32, tag="xT2",
                                name="xT2")
                nc.scalar.copy(xT2[:], psT2[:])

                psG = psG_p.tile([128, 2 * NI], F32, tag="psG", name="psG")
                nc.tensor.matmul(psG[:], xT1[:], e1_t[:],
                                 start=True, stop=False)
                nc.tensor.matmul(psG[:], xT2[:], e2_t[:],
                                 start=False, stop=False)
                nc.tensor.matmul(psG[:], ones_t[:], crow_t[:],
                                 start=False, stop=True)

                data_t = di_p.tile([128, NI], F16, tag="data", name="data")
                nc.vector.tensor_copy(data_t[:], psG[:, :NI])
                idxs_t = di_p.tile([128, NI], I16, tag="idxs", name="idxs")
                nc.scalar.copy(idxs_t[:], psG[:, NI:])

                out_t = out_p.tile([128, NE], F16, tag="out", name="out_t")
                nc.gpsimd.local_scatter(out_t[:], data_t[:], idxs_t[:],
                                        channels=128, num_elems=NE,
                                        num_idxs=NI)
                nc.sync.dma_start(out_sl[g], out_t[:])
            g0 += G

    nc.compile()
    return nc


_CACHE = {}


def _get_program(npad):
    if npad not in _CACHE:
        _CACHE[npad] = build_program(npad)
    return _CACHE[npad]


def run_sharded(x, trace=False):
    """x: [N, 3] fp32, N divisible by N_CORES. Returns ([N,195] f32, res)."""
    n = x.shape[0]
    assert n % N_CORES == 0
    nsh = n // N_CORES
    npad = int(math.ceil(nsh / GROUP)) * GROUP
    nc = _get_program(npad)
    ident, E1, E2, crow, ones = _host_consts()
    in_maps = []
    for i in range(N_CORES):
        sh = np.asarray(x[i * nsh:(i + 1) * nsh], dtype=np.float32)
        if npad != nsh:
            sh = np.concatenate(
                [sh, np.zeros((npad - nsh, D), np.float32)], axis=0)
        in_maps.append({
            "x": np.ascontiguousarray(sh),
            "ident": ident, "e1": E1, "e2": E2, "crow": crow, "ones": ones,
        })
    res = bass_utils.run_bass_kernel_spmd(
        nc, in_maps, core_ids=list(range(N_CORES)), trace=trace)
    outs = []
    for i in range(N_CORES):
        o = res.results[i]["out"]  # [npad, 195] f16
        outs.append(o[:nsh].astype(np.float32))
    return np.concatenate(outs, axis=0), res


def kernel(x):
    x = np.asarray(x, dtype=np.float32)
    out, _ = run_sharded(x, trace=False)
    return out


# revision 3
# speedup vs baseline: 1.7221x; 1.7221x over previous
"""Bass/Trainium2 kernel for cubic B-spline encoding — scatter arch, J=10, fp16 PE.

Per group: 1280 points = 128 partitions x 10 points.
  - DVE computes per-point features on [128, 30] tiles:
    xs=(x+1)*30.5, xh=xs-0.5, u=xs-idx, u^2, u^3
  - ScalarE casts xh -> int16 (round == floor(xs)) and back (idx feature).
  - PE transposes the [128, 150] feature tile in two halves and runs 3
    accumulating matmuls that emit, per (point, dim): the 4 spline
    coefficients c_q(u), the x passthrough, and the int16 scatter indices
    idx + rowbase + q (exact integers in fp32).
  - GPSIMD local_scatter zero-fills the [128, 10*195] fp16 out tile and
    places the 15 values per point.  No dense 64-wide elementwise work.
  - One contiguous DMA per group writes 10 rows x 195 fp16 per partition.
Output is fp16 in HBM (error ~2e-4 << 2e-2 tolerance); host upcasts to f32.
"""

import math
import os
import sys
from contextlib import ExitStack

import numpy as np

for _p in ("/opt/trn_rl_repo", "/root/.axon_site/_ro/trn_rl_repo"):
    if os.path.isdir(_p) and _p not in sys.path:
        sys.path.insert(0, _p)

import concourse.bass as bass  # noqa: E402,F401
import concourse.tile as tile  # noqa: E402
from concourse import bacc, mybir  # noqa: E402
from concourse import bass_utils  # noqa: E402

F32 = mybir.dt.float32
F16 = mybir.dt.float16
I16 = mybir.dt.int16

N_CORES = 8
D = 3
K = 64
ROW = D * (1 + K)          # 195 outputs per point
J = 10                     # points per partition per group
GROUP = 128 * J            # 1280 points per group
NB = J * D                 # 30 (j,d) blocks per partition per group
NF = 5                     # feature planes: u, u2, u3, x, idx
FROWS = NF * NB            # 150 feature rows (split 75/75 for transpose)
FH = 75                    # transpose half
NI = NB * 5                # 150 scatter slots (c0..c3 + x per (j,d))
NE = J * ROW               # 1950 out elems per partition (cap: 2047)
SCALE = (K - 3) / 2.0      # 30.5
MAX_SG = 12                # groups per input-DMA supergroup

PU, PU2, PU3, PX, PIDX = range(NF)


def _host_consts():
    ident = np.eye(128, dtype=np.float16)
    E = np.zeros((FROWS, 2 * NI), dtype=np.float64)
    crow = np.zeros((1, 2 * NI), dtype=np.float64)
    ones = np.ones((1, 128), dtype=np.float16)
    CQ = np.array([
        [1 / 6, -3 / 6, 3 / 6, -1 / 6],   # c0 = (1-u)^3/6
        [4 / 6, 0.0, -1.0, 0.5],          # c1 = (3u^3-6u^2+4)/6
        [1 / 6, 3 / 6, 3 / 6, -0.5],      # c2 = (-3u^3+3u^2+3u+1)/6
        [0.0, 0.0, 0.0, 1 / 6],           # c3 = u^3/6
    ], dtype=np.float64)
    for j in range(J):
        for d in range(D):
            m = j * D + d
            base = ROW * j + (1 + K) * d
            for q in range(4):
                c = m * 5 + q
                crow[0, c] = CQ[q, 0]
                E[PU * NB + m, c] = CQ[q, 1]
                E[PU2 * NB + m, c] = CQ[q, 2]
                E[PU3 * NB + m, c] = CQ[q, 3]
                E[PIDX * NB + m, NI + c] = 1.0
                crow[0, NI + c] = base + 1 + q
            cx = m * 5 + 4
            E[PX * NB + m, cx] = 1.0
            crow[0, NI + cx] = base
    E = E.astype(np.float16)
    crow = crow.astype(np.float16)
    return ident, np.ascontiguousarray(E[:FH]), np.ascontiguousarray(E[FH:]), crow, ones


def _split_supergroups(n_groups):
    sizes = []
    left = n_groups
    while left > 0:
        g = min(MAX_SG, left)
        sizes.append(g)
        left -= g
    return sizes


def build_program(npad):
    assert npad % GROUP == 0
    n_groups = npad // GROUP
    nc = bacc.Bacc("TRN2", target_bir_lowering=False, debug=False,
                   num_devices=N_CORES)
    x_d = nc.dram_tensor("x", [npad, D], F32, kind="ExternalInput").ap()
    out_d = nc.dram_tensor("out", [npad, ROW], F16,
                           kind="ExternalOutput").ap()
    ident_d = nc.dram_tensor("ident", [128, 128], F16,
                             kind="ExternalInput").ap()
    e1_d = nc.dram_tensor("e1", [FH, 2 * NI], F16, kind="ExternalInput").ap()
    e2_d = nc.dram_tensor("e2", [FROWS - FH, 2 * NI], F16,
                          kind="ExternalInput").ap()
    crow_d = nc.dram_tensor("crow", [1, 2 * NI], F16,
                            kind="ExternalInput").ap()
    ones_d = nc.dram_tensor("ones", [1, 128], F16, kind="ExternalInput").ap()

    AL = mybir.AluOpType

    with tile.TileContext(nc) as tc, ExitStack() as ctx:
        cpool = ctx.enter_context(tc.tile_pool(name="const", bufs=1))
        ident_t = cpool.tile([128, 128], F16, tag="ident")
        nc.sync.dma_start(ident_t[:], ident_d[:])
        e1_t = cpool.tile([FH, 2 * NI], F16, tag="e1")
        nc.sync.dma_start(e1_t[:], e1_d[:])
        e2_t = cpool.tile([FROWS - FH, 2 * NI], F16, tag="e2")
        nc.sync.dma_start(e2_t[:], e2_d[:])
        crow_t = cpool.tile([1, 2 * NI], F16, tag="crow")
        nc.sync.dma_start(crow_t[:], crow_d[:])
        ones_t = cpool.tile([1, 128], F16, tag="ones")
        nc.sync.dma_start(ones_t[:], ones_d[:])

        xin_p = ctx.enter_context(tc.tile_pool(name="xin", bufs=2))
        ft_p = ctx.enter_context(tc.tile_pool(name="ft", bufs=3))
        xs_p = ctx.enter_context(tc.tile_pool(name="xs", bufs=3))
        ii_p = ctx.enter_context(tc.tile_pool(name="ii", bufs=3))
        xT_p = ctx.enter_context(tc.tile_pool(name="xT", bufs=4))
        di_p = ctx.enter_context(tc.tile_pool(name="di", bufs=3))
        out_p = ctx.enter_context(tc.tile_pool(name="out", bufs=4))
        psT_p = ctx.enter_context(tc.tile_pool(name="psT", bufs=2,
                                               space="PSUM"))
        psG_p = ctx.enter_context(tc.tile_pool(name="psG", bufs=2,
                                               space="PSUM"))

        g0 = 0
        for G in _split_supergroups(n_groups):
            b0 = g0 * GROUP
            x_sl = x_d[b0:b0 + GROUP * G, :].rearrange(
                "(p k) d -> p (k d)", p=128)
            out_sl = out_d[b0:b0 + GROUP * G, :].rearrange(
                "(p g j) f -> g p (j f)", p=128, j=J)
            xin = xin_p.tile([128, G * NB], F32, tag="xin", name="xin")
            nc.sync.dma_start(xin[:], x_sl)

            for g in range(G):
                x_g = xin[:, g * NB:(g + 1) * NB]       # [128, 30]
                ft = ft_p.tile([128, FROWS], F16, tag="ft", name="ft")
                u = ft[:, PU * NB:(PU + 1) * NB]
                u2 = ft[:, PU2 * NB:(PU2 + 1) * NB]
                u3 = ft[:, PU3 * NB:(PU3 + 1) * NB]
                xf = ft[:, PX * NB:(PX + 1) * NB]
                idxf = ft[:, PIDX * NB:(PIDX + 1) * NB]

                xs = xs_p.tile([128, NB], F32, tag="xs", name="xs")
                # xs = (x + 1) * 30.5  (same op order as the reference)
                nc.vector.tensor_scalar(xs[:], x_g, 1.0, SCALE,
                                        AL.add, AL.mult)
                # idx = floor(xs) via round-to-nearest int16 cast of xs-0.5
                xh = xs_p.tile([128, NB], F32, tag="xh", name="xh")
                nc.vector.tensor_scalar(xh[:], xs[:], 0.5, 1.0,
                                        AL.subtract, AL.mult)
                idx_i = ii_p.tile([128, NB], I16, tag="ii", name="ii")
                nc.scalar.copy(idx_i[:], xh[:])
                idxf32 = xs_p.tile([128, NB], F32, tag="if32", name="if32")
                nc.scalar.copy(idxf32[:], idx_i[:])
                nc.scalar.copy(idxf, idx_i[:])
                nc.vector.tensor_tensor(u, xs[:], idxf32[:], AL.subtract)
                nc.vector.tensor_tensor(u2, u, u, AL.mult)
                nc.vector.tensor_tensor(u3, u2, u, AL.mult)
                nc.vector.tensor_copy(xf, x_g)

                psT1 = psT_p.tile([FH, 128], F16, tag="psT1", name="psT1")
                nc.tensor.transpose(psT1[:], ft[:, :FH], ident_t[:])
                psT2 = psT_p.tile([FROWS - FH, 128], F16, tag="psT2",
                                  name="psT2")
                nc.tensor.transpose(psT2[:], ft[:, FH:], ident_t[:])
                xT1 = xT_p.tile([FH, 128], F16, tag="xT1", name="xT1")
                nc.scalar.copy(xT1[:], psT1[:])
                xT2 = xT_p.tile([FROWS - FH, 128], F16, tag="xT2",
                                name="xT2")
                nc.scalar.copy(xT2[:], psT2[:])

                psG = psG_p.tile([128, 2 * NI], F32, tag="psG", name="psG")
                nc.tensor.matmul(psG[:], xT1[:], e1_t[:],
                                 start=True, stop=False)
                nc.tensor.matmul(psG[:], xT2[:], e2_t[:],
                                 start=False, stop=False)
                nc.tensor.matmul(psG[:], ones_t[:], crow_t[:],
                                 start=False, stop=True)

                data_t = di_p.tile([128, NI], F16, tag="data", name="data")
                nc.vector.tensor_copy(data_t[:], psG[:, :NI])
                idxs_t = di_p.tile([128, NI], I16, tag="idxs", name="idxs")
                nc.scalar.copy(idxs_t[:], psG[:, NI:])

                out_t = out_p.tile([128, NE], F16, tag="out", name="out_t")
                nc.gpsimd.local_scatter(out_t[:], data_t[:], idxs_t[:],
                                        channels=128, num_elems=NE,
                                        num_idxs=NI)
                nc.sync.dma_start(out_sl[g], out_t[:])
            g0 += G

    nc.compile()
    return nc


_CACHE = {}


def _get_program(npad):
    if npad not in _CACHE:
        _CACHE[npad] = build_program(npad)
    return _CACHE[npad]


def run_sharded(x, trace=False):
    """x: [N, 3] fp32, N divisible by N_CORES. Returns ([N,195] f32, res)."""
    n = x.shape[0]
    assert n % N_CORES == 0
    nsh = n // N_CORES
    npad = int(math.ceil(nsh / GROUP)) * GROUP
    nc = _get_program(npad)
    ident, E1, E2, crow, ones = _host_consts()
    in_maps = []
    for i in range(N_CORES):
        sh = np.asarray(x[i * nsh:(i + 1) * nsh], dtype=np.float32)
        if npad != nsh:
            sh = np.concatenate(
                [sh, np.zeros((npad - nsh, D), np.float32)], axis=0)
        in_maps.append({
            "x": np.ascontiguousarray(sh),
            "ident": ident, "e1": E1, "e2": E2, "crow": crow, "ones": ones,
        })
    res = bass_utils.run_bass_kernel_spmd(
        nc, in_maps, core_ids=list(range(N_CORES)), trace=trace)
    outs = []
    for i in range(N_CORES):
        o = res.results[i]["out"]  # [npad, 195] f16
        outs.append(o[:nsh].astype(np.float32))
    return np.concatenate(outs, axis=0), res


def kernel(x):
    x = np.asarray(x, dtype=np.float32)
    out, _ = run_sharded(x, trace=False)
    return out
